# revision 9
# baseline (speedup 1.0000x reference)
"""Self-contained Trainium2 Bass kernel for nn_ASMLoc (sparse attention over video proposals).

Sharding: pure data-parallel over batch B=16 across 8 NeuronCores (2 videos/core).
All matmuls bf16 with f32 PSUM accumulation; residual trunk kept f32 on-chip.
"""
import numpy as np
import ml_dtypes

import concourse.bass as bass
import concourse.mybir as mybir
from concourse import tile
from concourse.bass_utils import run_bass_kernel_spmd
from concourse.masks import make_identity
from concourse.vector_clock import ScopedClock

# ---------------- problem constants (hardcoded per spec) ----------------
B, T, C, K, HEADS = 16, 750, 2048, 50, 8
NCLS = 21
DH = C // HEADS           # 256
SCALE = DH ** -0.5
FG_TOPK = max(T // 8, 1)  # 93
BG_TOPK = max(T // 3, 1)  # 250
NCORES = 8
V = B // NCORES           # videos per core
TP = 768                  # padded T (6 x 128)
NCH = 16                  # C/128 chunks
NT = 6                    # TP/128 tiles
F32 = mybir.dt.float32
BF16 = mybir.dt.bfloat16
AF = mybir.ActivationFunctionType
Alu = mybir.AluOpType
AX = mybir.AxisListType
NEG = -1e30
# per-partition column table indices
EMB_B, QB_I, KB_I, VB_I, SC_I, SH_I, QB_E, KB_E, VB_E, SC_E, SH_E = range(11)
NPP = 11


# ---------------- Tile/walrus workarounds ----------------
# This walrus build supports at most ONE sync wait per instruction. Tile's
# scheduler attaches several. Hoist the excess onto standalone wait_ge
# instructions inserted immediately before the owner on the same engine.
def _hoist_excess_waits(self):
    nc = self.nc
    by_num = {h.num: h for h in self.sems.allocated().values()}
    cur_list = nc.cur_bb.bb.instructions
    for blk in nc.m.functions[0].blocks:
        insts = blk.instructions
        i = 0
        while i < len(insts):
            inst = insts[i]
            si = inst.sync_info
            waits = list(si.on_wait) if si is not None and si.on_wait else []
            if len(waits) > 1:
                si.on_wait = [waits[0]]
                new_insts = []
                for w in waits[1:]:
                    assert w.wait_reg is None, f"register wait on {inst.name}"
                    wi = nc.engines[inst.engine].wait_ge(by_num[w.id], w.wait_value)
                    popped = cur_list.pop()
                    assert popped.name == wi.ins.name
                    new_insts.append(popped)
                for j, wmi in enumerate(new_insts):
                    insts.insert(i + j, wmi)
                i += len(new_insts)
            i += 1


def _patched_drain_and_barrier(self, tick_clock, wait_clock):
    _hoist_excess_waits(self)
    nc = self.nc
    carrier = nc.sync.nop()
    wait_clock.add_sem_waits(carrier.ins, ScopedClock({None: tick_clock.global_clock}))
    si = carrier.ins.sync_info
    waits = list(si.on_wait or []) if si is not None else []
    if waits:
        by_num = {h.num: h for h in self.sems.allocated().values()}
        si.on_wait = []
        for w in waits:
            nc.sync.wait_ge(by_num[w.id], w.wait_value)
    nc.sync.drain()
    nc.all_engine_barrier()
    assert self.sems is not None
    popped = nc._tile_sem_poison_stack.pop()
    assert popped is self._sem_poison
    nc.clear_and_free_semaphores(list(self.sems.allocated().values()))
    nc.all_engine_barrier()


tile.TileContext._drain_and_barrier = _patched_drain_and_barrier


# ---------------- program builder ----------------
def build_program():
    nc = bass.Bass()
    d = {}
    d["xt"] = nc.dram_tensor("xt", [V, NCH, 128, TP + 2], BF16, kind="ExternalInput")
    d["w_emb"] = nc.dram_tensor("w_emb", [NCH, 128, 3, NCH, 128], BF16, kind="ExternalInput")
    for n in ("wq_i", "wk_i", "wv_i", "wo_i", "wq_e", "wk_e", "wv_e", "wo_e"):
        d[n] = nc.dram_tensor(n, [NCH, 128, NCH, 128], BF16, kind="ExternalInput")
    d["cls_wr"] = nc.dram_tensor("cls_wr", [NCH, 128, NCLS], BF16, kind="ExternalInput")
    d["att_wr"] = nc.dram_tensor("att_wr", [NCH, 128, 2], BF16, kind="ExternalInput")
    d["unc_wr"] = nc.dram_tensor("unc_wr", [NCH, 128, 1], BF16, kind="ExternalInput")
    d["pp"] = nc.dram_tensor("pp", [128, NPP * NCH], F32, kind="ExternalInput")
    d["srow"] = nc.dram_tensor("srow", [1, 32], F32, kind="ExternalInput")
    d["tio"] = nc.dram_tensor("tio", [1, TP], F32, kind="ExternalInput")
    d["prop"] = nc.dram_tensor("prop", [V, 50, 3], F32, kind="ExternalInput")
    d["vrow"] = nc.dram_tensor("vrow", [V, 1, 50], F32, kind="ExternalInput")
    d["kinv"] = nc.dram_tensor("kinv", [64, 1], F32, kind="ExternalInput")
    d["o_fg"] = nc.dram_tensor("o_fg", [V, NCLS], F32, kind="ExternalOutput")
    d["o_bg"] = nc.dram_tensor("o_bg", [V, NCLS], F32, kind="ExternalOutput")
    d["o_ta"] = nc.dram_tensor("o_ta", [V, TP, 2], F32, kind="ExternalOutput")
    d["o_cas"] = nc.dram_tensor("o_cas", [V, TP, NCLS], F32, kind="ExternalOutput")
    d["o_fgs"] = nc.dram_tensor("o_fgs", [V, TP, NCLS], F32, kind="ExternalOutput")
    d["o_bgs"] = nc.dram_tensor("o_bgs", [V, TP, NCLS], F32, kind="ExternalOutput")
    d["o_unc"] = nc.dram_tensor("o_unc", [V, TP, 1], F32, kind="ExternalOutput")

    with tile.TileContext(nc) as tc:
        _emit(nc, tc, d)
    nc.finalize()
    return nc


def _mm(nc, ps, pairs):
    n = len(pairs)
    for j, (l, r) in enumerate(pairs):
        nc.tensor.matmul(ps, l, r, start=(j == 0), stop=(j == n - 1))


class Ctx:
    pass


def _emit(nc, tc, d):
    from contextlib import ExitStack

    with ExitStack() as root:
        g = Ctx()
        g.nc, g.tc, g.d = nc, tc, d
        cpool = root.enter_context(tc.tile_pool(name="const", bufs=1))
        g.ppool = root.enter_context(tc.tile_pool(name="ps", bufs=8, space="PSUM"))

        def psum(pw=512, dt=F32):
            return g.ppool.tile([128, pw], dt, tag="ps", name=f"ps{pw}")

        g.psum = psum
        g.id_bf = cpool.tile([128, 128], BF16, tag="id_bf")
        make_identity(nc, g.id_bf)
        g.id_f = cpool.tile([128, 128], F32, tag="id_f")
        make_identity(nc, g.id_f)
        g.ones = cpool.tile([128, 128], F32, tag="ones")
        nc.vector.memset(g.ones, 1.0)
        g.onesb = cpool.tile([128, 128], BF16, tag="onesb")
        nc.vector.memset(g.onesb, 1.0)
        pp = cpool.tile([128, NPP * NCH], F32, tag="pp")
        nc.sync.dma_start(pp, d["pp"][:])
        g.ppcol = lambda idx, oc: pp[:, idx * NCH + oc: idx * NCH + oc + 1]
        g.srow = cpool.tile([1, 32], F32, tag="srow")
        nc.sync.dma_start(g.srow, d["srow"][:])
        tio = cpool.tile([1, TP], F32, tag="tio")
        nc.sync.dma_start(tio, d["tio"][:])
        g.kinv = cpool.tile([64, 1], F32, tag="kinv")
        nc.sync.dma_start(g.kinv, d["kinv"][:])
        # t broadcast [50, TP]
        g.t_b = cpool.tile([50, TP], F32, tag="t_b")
        for mh in range(2):
            ps = psum(384)
            _mm(nc, ps[:50, :384], [(g.ones[0:1, 0:50], tio[:, mh * 384:(mh + 1) * 384])])
            nc.scalar.copy(g.t_b[:, mh * 384:(mh + 1) * 384], ps[:50, :384])

        g.bpool = root.enter_context(tc.tile_pool(name="big", bufs=1))
        for v in range(V):
            _video(g, v)


def _video(g, v):
    from contextlib import ExitStack
    nc, tc, d, psum = g.nc, g.tc, g.d, g.psum
    ppcol = g.ppcol

    trunk = g.bpool.tile([128, NCH, TP], F32, tag="trunk")
    emb_bf = g.bpool.tile([128, NCH, TP], BF16, tag="emb_bf")
    mask01 = g.bpool.tile([128, NT, TP], BF16, tag="mask01")
    Prn = g.bpool.tile([50, TP], BF16, tag="Prn")
    Pn = g.bpool.tile([128, NT, 50], BF16, tag="Pn")
    inter01 = g.bpool.tile([50, 50], BF16, tag="inter01")
    casT = g.bpool.tile([64, TP], F32, tag="casT")

    # ---------------- masks ----------------
    with tc.tile_pool(name="mb", bufs=1) as mb:
        se = mb.tile([50, 3], F32, tag="se")
        nc.sync.dma_start(se, d["prop"][v])
        vrow = mb.tile([1, 50], F32, tag="vrow")
        nc.sync.dma_start(vrow, d["vrow"][v])
        PT = mb.tile([50, TP], F32, tag="PT")
        PT_bf = mb.tile([50, TP], BF16, tag="PT_bf")
        tmp = mb.tile([50, TP], F32, tag="mtmp")
        nc.vector.tensor_scalar(PT, g.t_b[:50], se[:, 0:1], None, op0=Alu.is_ge)
        nc.vector.tensor_scalar(tmp[:50], g.t_b[:50], se[:, 1:2], None, op0=Alu.is_le)
        nc.vector.tensor_tensor(PT, PT, tmp[:50], op=Alu.mult)
        nc.vector.tensor_scalar(PT, PT, se[:, 2:3], None, op0=Alu.mult)
        nc.vector.tensor_copy(PT_bf, PT)
        cs_t = mb.tile([50, 1], F32, tag="cs_t")
        nc.vector.reduce_sum(cs_t, PT, axis=AX.X)
        nc.vector.tensor_scalar_max(cs_t, cs_t, 1e-5)
        nc.vector.reciprocal(cs_t, cs_t)
        ptn = mb.tile([50, TP], BF16, tag="ptn")
        nc.vector.tensor_scalar(ptn[:50], PT, cs_t, None, op0=Alu.mult)
        for ti in range(NT):
            ps = psum(128, BF16)
            nc.tensor.transpose(ps[:128, :50], ptn[:50, ti * 128:(ti + 1) * 128], g.id_bf[:50, :50])
            nc.scalar.copy(Pn[:, ti, :], ps[:128, :50])
        rk = mb.tile([1, TP], F32, tag="rk")
        for mh in range(2):
            sl = slice(mh * 384, (mh + 1) * 384)
            ps = psum(384)
            _mm(nc, ps[:1, :384], [(g.onesb[0:50, 0:1], PT_bf[:, sl])])
            nc.scalar.copy(rk[:, sl], ps[:1, :384])
        nc.vector.tensor_scalar_max(rk, rk, 1e-5)
        nc.vector.reciprocal(rk, rk)
        for mh in range(2):
            sl = slice(mh * 384, (mh + 1) * 384)
            ps = psum(384)
            _mm(nc, ps[:50, :384], [(g.ones[0:1, 0:50], rk[:, sl])])
            nc.vector.tensor_tensor(Prn[:, sl], ps[:50, :384], PT_bf[:, sl], op=Alu.mult)
        for nt in range(NT):
            for mh in range(2):
                ps = psum(384)
                _mm(nc, ps[:, :384], [(PT_bf[:, nt * 128:(nt + 1) * 128], PT_bf[:, mh * 384:(mh + 1) * 384])])
                nc.vector.tensor_scalar(mask01[:, nt, mh * 384:(mh + 1) * 384], ps[:, :384], 0.0, None, op0=Alu.is_gt)
        ps = psum(50)
        _mm(nc, ps[:50, :50], [(vrow, vrow)])
        nc.vector.tensor_scalar(inter01, ps[:50, :50], 0.0, None, op0=Alu.is_gt)

    # ---------------- conv embedding ----------------
    with tc.tile_pool(name="cva", bufs=1) as cva, tc.tile_pool(name="cvw", bufs=2) as cvw:
        x_sb = cva.tile([128, NCH, TP + 2], BF16, tag="x")
        for ic in range(NCH):
            nc.sync.dma_start(x_sb[:, ic, :], d["xt"][v, ic])
        for ot in range(NCH):
            wt = cvw.tile([128, 3, NCH, 128], BF16, tag="wem")
            nc.sync.dma_start(wt, d["w_emb"][ot])
            for nh in range(2):
                ps = psum(384)
                pairs = []
                for dd in range(3):
                    for ic in range(NCH):
                        pairs.append((wt[:, dd, ic, :], x_sb[:, ic, nh * 384 + dd: nh * 384 + dd + 384]))
                _mm(nc, ps[:, :384], pairs)
                nc.scalar.activation(trunk[:, ot, nh * 384:(nh + 1) * 384], ps[:, :384], AF.Relu, bias=ppcol(EMB_B, ot))
    nc.vector.tensor_copy(emb_bf, trunk)

    # ---------------- intra attention ----------------
    with ExitStack() as st:
        apool = st.enter_context(tc.tile_pool(name="att", bufs=1))
        wpool = st.enter_context(tc.tile_pool(name="attw", bufs=2))
        hpool = st.enter_context(tc.tile_pool(name="atth", bufs=2))
        npool = st.enter_context(tc.tile_pool(name="attn", bufs=3))
        vT = apool.tile([128, NT, C], BF16, tag="vT")
        out_sb = apool.tile([128, NCH, TP], BF16, tag="out_sb")
        attnT = apool.tile([128, NT, TP], BF16, tag="attnT")

        # v channel-major then PE-transpose into vT [t, o]
        for ot in range(NCH):
            wv = wpool.tile([128, NCH, 128], BF16, tag="w")
            nc.sync.dma_start(wv, d["wv_i"][ot])
            vst = npool.tile([128, TP], BF16, tag="vst")
            for nh in range(2):
                ps = psum(384)
                _mm(nc, ps[:, :384], [(wv[:, ic, :], emb_bf[:, ic, nh * 384:(nh + 1) * 384]) for ic in range(NCH)])
                nc.scalar.activation(vst[:, nh * 384:(nh + 1) * 384], ps[:, :384], AF.Identity, bias=ppcol(VB_I, ot))
            for ti in range(NT):
                pt = psum(128, BF16)
                nc.tensor.transpose(pt[:, :128], vst[:, ti * 128:(ti + 1) * 128], g.id_bf)
                nc.scalar.copy(vT[:, ti, ot * 128:(ot + 1) * 128], pt[:, :128])

        for h in range(HEADS):
            q_h = hpool.tile([128, 2, TP], BF16, tag="q_h")
            k_h = hpool.tile([128, 2, TP], BF16, tag="k_h")
            for dc in range(2):
                oc = 2 * h + dc
                wq = wpool.tile([128, NCH, 128], BF16, tag="w")
                nc.sync.dma_start(wq, d["wq_i"][oc])
                wk = wpool.tile([128, NCH, 128], BF16, tag="w")
                nc.sync.dma_start(wk, d["wk_i"][oc])
                for nh in range(2):
                    sl = slice(nh * 384, (nh + 1) * 384)
                    ps = psum(384)
                    _mm(nc, ps[:, :384], [(wq[:, ic, :], emb_bf[:, ic, sl]) for ic in range(NCH)])
                    nc.scalar.activation(q_h[:, dc, sl], ps[:, :384], AF.Identity, bias=ppcol(QB_I, oc), scale=SCALE)
                    ps = psum(384)
                    _mm(nc, ps[:, :384], [(wk[:, ic, :], emb_bf[:, ic, sl]) for ic in range(NCH)])
                    nc.scalar.activation(k_h[:, dc, sl], ps[:, :384], AF.Identity, bias=ppcol(KB_I, oc))
            for nt in range(NT):
                nsl = slice(nt * 128, (nt + 1) * 128)
                sa = psum(384)
                sb_ = psum(384)
                _mm(nc, sa[:, :384], [(q_h[:, dc, nsl], k_h[:, dc, 0:384]) for dc in range(2)])
                _mm(nc, sb_[:, :384], [(q_h[:, dc, nsl], k_h[:, dc, 384:768]) for dc in range(2)])
                attn = npool.tile([128, TP], BF16, tag="attn")
                _softmax_masked(g, npool, sa[:, :384], sb_[:, :384], mask01[:, nt, :], attn)
                for mc in range(NT):
                    pt = psum(128, BF16)
                    nc.tensor.transpose(pt[:, :128], attn[:, mc * 128:(mc + 1) * 128], g.id_bf)
                    nc.scalar.copy(attnT[:, mc, nsl], pt[:, :128])
            for dc in range(2):
                oc = 2 * h + dc
                for nh in range(2):
                    sl = slice(nh * 384, (nh + 1) * 384)
                    ps = psum(384)
                    _mm(nc, ps[:, :384], [(vT[:, mc, oc * 128:(oc + 1) * 128], attnT[:, mc, sl]) for mc in range(NT)])
                    nc.scalar.copy(out_sb[:, oc, sl], ps[:, :384])

        for ot in range(NCH):
            wo = wpool.tile([128, NCH, 128], BF16, tag="w")
            nc.sync.dma_start(wo, d["wo_i"][ot])
            for nh in range(2):
                sl = slice(nh * 384, (nh + 1) * 384)
                ps = psum(384)
                _mm(nc, ps[:, :384], [(wo[:, ic, :], out_sb[:, ic, sl]) for ic in range(NCH)])
                tmp = npool.tile([128, 384], F32, tag="res")
                nc.scalar.activation(tmp, ps[:, :384], AF.Identity, bias=ppcol(SH_I, ot), scale=ppcol(SC_I, ot))
                nc.vector.tensor_tensor(trunk[:, ot, sl], trunk[:, ot, sl], tmp, op=Alu.add)
    nc.vector.tensor_copy(emb_bf, trunk)

    # ---------------- inter attention ----------------
    with ExitStack() as st:
        epool = st.enter_context(tc.tile_pool(name="ep", bufs=1))
        ewp = st.enter_context(tc.tile_pool(name="ewp", bufs=2))
        esp = st.enter_context(tc.tile_pool(name="esp", bufs=3))
        embT = epool.tile([128, NT, C], BF16, tag="embT")
        for ic in range(NCH):
            for ti in range(NT):
                pt = psum(128, BF16)
                nc.tensor.transpose(pt[:, :128], emb_bf[:, ic, ti * 128:(ti + 1) * 128], g.id_bf)
                nc.scalar.copy(embT[:, ti, ic * 128:(ic + 1) * 128], pt[:, :128])
        seg = epool.tile([128, NCH, 50], BF16, tag="seg")
        for ct in range(NCH):
            ps = psum(50)
            _mm(nc, ps[:, :50], [(embT[:, ti, ct * 128:(ct + 1) * 128], Pn[:, ti, :]) for ti in range(NT)])
            nc.scalar.copy(seg[:, ct, :], ps[:, :50])
        q_e = epool.tile([128, NCH, 50], BF16, tag="q_e")
        k_e = epool.tile([128, NCH, 50], BF16, tag="k_e")
        veT = epool.tile([50, C], BF16, tag="veT")
        oogT = epool.tile([50, C], BF16, tag="oogT")
        for ot in range(NCH):
            wq = ewp.tile([128, NCH, 128], BF16, tag="we")
            nc.sync.dma_start(wq, d["wq_e"][ot])
            wk = ewp.tile([128, NCH, 128], BF16, tag="we")
            nc.sync.dma_start(wk, d["wk_e"][ot])
            wv = ewp.tile([128, NCH, 128], BF16, tag="we")
            nc.sync.dma_start(wv, d["wv_e"][ot])
            ps = psum(50)
            _mm(nc, ps[:, :50], [(wq[:, ic, :], seg[:, ic, :]) for ic in range(NCH)])
            nc.scalar.activation(q_e[:, ot, :], ps[:, :50], AF.Identity, bias=ppcol(QB_E, ot), scale=SCALE)
            ps = psum(50)
            _mm(nc, ps[:, :50], [(wk[:, ic, :], seg[:, ic, :]) for ic in range(NCH)])
            nc.scalar.activation(k_e[:, ot, :], ps[:, :50], AF.Identity, bias=ppcol(KB_E, ot))
            ps = psum(50)
            _mm(nc, ps[:, :50], [(wv[:, ic, :], seg[:, ic, :]) for ic in range(NCH)])
            vst = esp.tile([128, 64], BF16, tag="vste")
            nc.scalar.activation(vst[:, :50], ps[:, :50], AF.Identity, bias=ppcol(VB_E, ot))
            pt = psum(128, BF16)
            nc.tensor.transpose(pt[:50, :128], vst[:, :50], g.id_bf)
            nc.scalar.copy(veT[:, ot * 128:(ot + 1) * 128], pt[:50, :128])
        outg = epool.tile([128, NCH, 50], BF16, tag="outg")
        for h in range(HEADS):
            ps = psum(50)
            _mm(nc, ps[:50, :50], [(q_e[:, 2 * h + dc, :], k_e[:, 2 * h + dc, :]) for dc in range(2)])
            attn_e = esp.tile([50, 50], BF16, tag="attn_e")
            _softmax_masked_small(g, esp, ps[:50, :50], inter01, attn_e)
            pt = psum(50, BF16)
            nc.tensor.transpose(pt[:50, :50], attn_e, g.id_bf[:50, :50])
            aet = esp.tile([50, 50], BF16, tag="aet")
            nc.scalar.copy(aet, pt[:50, :50])
            for dc in range(2):
                oc = 2 * h + dc
                ps = psum(50)
                _mm(nc, ps[:, :50], [(veT[:, oc * 128:(oc + 1) * 128], aet)])
                nc.scalar.copy(outg[:, oc, :], ps[:, :50])
        for ot in range(NCH):
            wo = ewp.tile([128, NCH, 128], BF16, tag="we")
            nc.sync.dma_start(wo, d["wo_e"][ot])
            ps = psum(50)
            _mm(nc, ps[:, :50], [(wo[:, ic, :], outg[:, ic, :]) for ic in range(NCH)])
            vst = esp.tile([128, 64], BF16, tag="ogst")
            nc.scalar.copy(vst[:, :50], ps[:, :50])
            pt = psum(128, BF16)
            nc.tensor.transpose(pt[:50, :128], vst[:, :50], g.id_bf)
            nc.scalar.copy(oogT[:, ot * 128:(ot + 1) * 128], pt[:50, :128])
        for ot in range(NCH):
            for nh in range(2):
                sl = slice(nh * 384, (nh + 1) * 384)
                ps = psum(384)
                _mm(nc, ps[:, :384], [(oogT[:, ot * 128:(ot + 1) * 128], Prn[:, sl])])
                tmp = esp.tile([128, 384], F32, tag="res_e")
                nc.scalar.activation(tmp, ps[:, :384], AF.Identity, bias=ppcol(SH_E, ot), scale=ppcol(SC_E, ot))
                nc.vector.tensor_tensor(trunk[:, ot, sl], trunk[:, ot, sl], tmp, op=Alu.add)
    nc.vector.tensor_copy(emb_bf, trunk)

    # ---------------- heads ----------------
    nc.vector.memset(casT, NEG)
    with tc.tile_pool(name="hd", bufs=3) as hd:
        cls_w = hd.tile([128, NCH, NCLS], BF16, tag="cls_w")
        nc.sync.dma_start(cls_w, d["cls_wr"][:].rearrange("c p n -> p c n"))
        att_w = hd.tile([128, NCH, 2], BF16, tag="att_w")
        nc.sync.dma_start(att_w, d["att_wr"][:].rearrange("c p n -> p c n"))
        unc_w = hd.tile([128, NCH, 1], BF16, tag="unc_w")
        nc.sync.dma_start(unc_w, d["unc_wr"][:].rearrange("c p n -> p c n"))
        for tt in range(NT):
            tsl = slice(tt * 128, (tt + 1) * 128)
            ps = psum(2)
            _mm(nc, ps[:, :2], [(emb_bf[:, ic, tsl], att_w[:, ic, :]) for ic in range(NCH)]
                + [(g.ones[0:1, 0:128], g.srow[:, 21:23])])
            ta = hd.tile([128, 2], F32, tag="ta")
            _softmax_row(g, hd, ps[:, :2], ta, 2)
            nc.sync.dma_start(d["o_ta"][v, tsl, :], ta)
            ps = psum(1)
            _mm(nc, ps[:, :1], [(emb_bf[:, ic, tsl], unc_w[:, ic, :]) for ic in range(NCH)])
            unc = hd.tile([128, 1], F32, tag="unc")
            nc.scalar.copy(unc, ps[:, :1])
            nc.sync.dma_start(d["o_unc"][v, tsl, :], unc)
            ps = psum(NCLS)
            _mm(nc, ps[:, :NCLS], [(emb_bf[:, ic, tsl], cls_w[:, ic, :]) for ic in range(NCH)]
                + [(g.ones[0:1, 0:128], g.srow[:, 0:21])])
            cas = hd.tile([128, NCLS], F32, tag="cas")
            nc.scalar.copy(cas, ps[:, :NCLS])
            fg = hd.tile([128, NCLS], F32, tag="fg")
            bg = hd.tile([128, NCLS], F32, tag="bg")
            nc.vector.tensor_scalar(fg, cas, ta[:, 0:1], None, op0=Alu.mult)
            nc.vector.tensor_scalar(bg, cas, ta[:, 1:2], None, op0=Alu.mult)
            for src, dst in ((cas, "o_cas"), (fg, "o_fgs"), (bg, "o_bgs")):
                sm = hd.tile([128, NCLS], F32, tag="sm")
                _softmax_row(g, hd, src, sm, NCLS)
                nc.sync.dma_start(d[dst][v, tsl, :], sm)
            for src, ro in ((fg, 0), (bg, 32)):
                pt = psum(128)
                nc.tensor.transpose(pt[:NCLS, :128], src[:, :NCLS], g.id_f)
                nc.scalar.copy(casT[ro:ro + NCLS, tsl], pt[:NCLS, :128])
    nc.vector.memset(casT[:, T:TP], NEG)

    # ---------------- top-k means + final softmax ----------------
    with tc.tile_pool(name="tk", bufs=1) as tk:
        acc = tk.tile([64, 1], F32, tag="acc")
        nc.vector.memset(acc, 0.0)
        m8 = tk.tile([64, 8], F32, tag="m8")
        red = tk.tile([64, 1], F32, tag="red")
        for p in range(32):
            nc.vector.max(out=m8, in_=casT)
            nc.vector.match_replace(out=casT, in_to_replace=m8, in_values=casT, imm_value=NEG)
            if p < 11:
                nc.vector.reduce_sum(red, m8, axis=AX.X)
                nc.vector.tensor_tensor(acc, acc, red, op=Alu.add)
            elif p == 11:
                nc.vector.reduce_sum(red[0:21], m8[0:21, 0:5], axis=AX.X)
                nc.vector.reduce_sum(red[32:53], m8[32:53, :], axis=AX.X)
                nc.vector.tensor_tensor(acc, acc, red, op=Alu.add)
            elif p < 31:
                nc.vector.reduce_sum(red[32:53], m8[32:53, :], axis=AX.X)
                nc.vector.tensor_tensor(acc[32:53], acc[32:53], red[32:53], op=Alu.add)
            else:
                nc.vector.reduce_sum(red[32:53], m8[32:53, 0:2], axis=AX.X)
                nc.vector.tensor_tensor(acc[32:53], acc[32:53], red[32:53], op=Alu.add)
        nc.vector.tensor_scalar(acc, acc, g.kinv, None, op0=Alu.mult)
        pt = psum(64)
        nc.tensor.transpose(pt[:1, :64], acc[:, 0:1], g.id_f[:64, :64])
        accT = tk.tile([1, 64], F32, tag="accT")
        nc.scalar.copy(accT, pt[:1, :64])
        for ro, dst in ((0, "o_fg"), (32, "o_bg")):
            sm = tk.tile([1, NCLS], F32, tag="smT")
            _softmax_row(g, tk, accT[:, ro:ro + NCLS], sm, NCLS, part=1)
            nc.sync.dma_start(d[dst][v:v + 1, :], sm)


def _softmax_masked(g, pool, sa, sb_, mask, attn):
    nc = g.nc
    ra = pool.tile([128, 1], F32, tag="ra")
    rb = pool.tile([128, 1], F32, tag="rb")
    nc.vector.reduce_max(ra, sa, axis=AX.X)
    nc.vector.reduce_max(rb, sb_, axis=AX.X)
    nc.vector.tensor_tensor(ra, ra, rb, op=Alu.max)
    nc.vector.tensor_scalar(ra, ra, -1.0, None, op0=Alu.mult)
    nc.scalar.activation(attn[:, 0:384], sa, AF.Exp, bias=ra)
    nc.scalar.activation(attn[:, 384:768], sb_, AF.Exp, bias=ra)
    rsa = pool.tile([128, 1], F32, tag="rsa")
    rsb = pool.tile([128, 1], F32, tag="rsb")
    nc.vector.scalar_tensor_tensor(attn[:, 0:384], attn[:, 0:384], 1.0, mask[:, 0:384],
                                   op0=Alu.mult, op1=Alu.mult, accum_out=rsa)
    nc.vector.scalar_tensor_tensor(attn[:, 384:768], attn[:, 384:768], 1.0, mask[:, 384:768],
                                   op0=Alu.mult, op1=Alu.mult, accum_out=rsb)
    nc.vector.tensor_tensor(rsa, rsa, rsb, op=Alu.add)
    nc.vector.tensor_scalar_max(rsa, rsa, 1e-30)
    nc.vector.reciprocal(rsa, rsa)
    nc.vector.tensor_scalar(attn[:, 0:384], attn[:, 0:384], rsa, None, op0=Alu.mult)
    nc.vector.tensor_scalar(attn[:, 384:768], attn[:, 384:768], rsa, None, op0=Alu.mult)


def _softmax_masked_small(g, pool, ps, mask, attn):
    nc = g.nc
    r = pool.tile([50, 1], F32, tag="rme")
    nc.vector.reduce_max(r, ps, axis=AX.X)
    nc.vector.tensor_scalar(r, r, -1.0, None, op0=Alu.mult)
    nc.scalar.activation(attn, ps, AF.Exp, bias=r)
    rs = pool.tile([50, 1], F32, tag="rse")
    nc.vector.scalar_tensor_tensor(attn, attn, 1.0, mask, op0=Alu.mult, op1=Alu.mult, accum_out=rs)
    nc.vector.tensor_scalar_max(rs, rs, 1e-30)
    nc.vector.reciprocal(rs, rs)
    nc.vector.tensor_scalar(attn, attn, rs, None, op0=Alu.mult)


def _softmax_row(g, pool, src, dst, n, part=128):
    nc = g.nc
    r = pool.tile([part, 1], F32, tag=f"smr{part}")
    nc.vector.reduce_max(r, src, axis=AX.X)
    nc.vector.tensor_scalar(r, r, -1.0, None, op0=Alu.mult)
    rs = pool.tile([part, 1], F32, tag=f"smrs{part}")
    nc.scalar.activation(dst, src, AF.Exp, bias=r, accum_out=rs)
    nc.vector.reciprocal(rs, rs)
    nc.vector.tensor_scalar(dst, dst, rs, None, op0=Alu.mult)


# ---------------- host side ----------------
_NC_CACHE = {}
_LAST_IN_MAPS = None


def _get_nc():
    if "nc" not in _NC_CACHE:
        _NC_CACHE["nc"] = build_program()
    return _NC_CACHE["nc"]


def _lhsT_layout(w):
    # W [O, I] -> [16(ot), 128(ip), 16(ic), 128(o)]: tile[ot][p, ic, o] = W[ot*128+o, ic*128+p]
    return np.ascontiguousarray(
        w.T.reshape(NCH, 128, NCH, 128).transpose(2, 1, 0, 3)).astype(ml_dtypes.bfloat16)


def _pp_layout(x):
    return np.ascontiguousarray(x.reshape(NCH, 128).T).astype(np.float32)


def kernel(**inp):
    inp = {k: np.asarray(v) for k, v in inp.items()}
    nc = _get_nc()

    emb_w = inp["emb_w"].astype(np.float32)  # [O, I, 3]
    # [ot, ip, d, ic, o]: element = emb_w[ot*128+o, ic*128+ip, d]
    w_emb = np.ascontiguousarray(
        emb_w.reshape(NCH, 128, NCH, 128, 3).transpose(0, 3, 4, 2, 1)
    ).astype(ml_dtypes.bfloat16)

    def bn_fold(pre):
        gg, b_ = inp[pre + "bng"], inp[pre + "bnb"]
        m, vv = inp[pre + "bnm"], inp[pre + "bnv"]
        sc = gg / np.sqrt(vv + 1e-5)
        sh = (inp[pre + "ob"] - m) * sc + b_
        return sc.astype(np.float32), sh.astype(np.float32)

    sc_i, sh_i = bn_fold("i_")
    sc_e, sh_e = bn_fold("e_")
    pp = np.stack([
        _pp_layout(inp["emb_b"]),
        _pp_layout(inp["i_qb"] * SCALE), _pp_layout(inp["i_kb"]), _pp_layout(inp["i_vb"]),
        _pp_layout(sc_i), _pp_layout(sh_i),
        _pp_layout(inp["e_qb"] * SCALE), _pp_layout(inp["e_kb"]), _pp_layout(inp["e_vb"]),
        _pp_layout(sc_e), _pp_layout(sh_e),
    ]).astype(np.float32)
    pp = np.ascontiguousarray(pp.transpose(1, 0, 2).reshape(128, NPP * NCH))
    srow = np.zeros((1, 32), np.float32)
    srow[0, :21] = inp["cls_b"]
    srow[0, 21:23] = inp["att_b"]
    tio = np.full((1, TP), -1.0, np.float32)
    tio[0, :T] = np.arange(T, dtype=np.float32)
    kinv = np.ones((64, 1), np.float32)
    kinv[0:NCLS, 0] = 1.0 / FG_TOPK
    kinv[32:32 + NCLS, 0] = 1.0 / BG_TOPK

    shared = {
        "w_emb": w_emb,
        "wq_i": _lhsT_layout(inp["i_qw"]), "wk_i": _lhsT_layout(inp["i_kw"]),
        "wv_i": _lhsT_layout(inp["i_vw"]), "wo_i": _lhsT_layout(inp["i_ow"]),
        "wq_e": _lhsT_layout(inp["e_qw"]), "wk_e": _lhsT_layout(inp["e_kw"]),
        "wv_e": _lhsT_layout(inp["e_vw"]), "wo_e": _lhsT_layout(inp["e_ow"]),
        "cls_wr": np.ascontiguousarray(inp["cls_w"].reshape(NCH, 128, NCLS)).astype(ml_dtypes.bfloat16),
        "att_wr": np.ascontiguousarray(inp["att_w"].T.reshape(NCH, 128, 2)).astype(ml_dtypes.bfloat16),
        "unc_wr": np.ascontiguousarray(inp["unc_w"].T.reshape(NCH, 128, 1)).astype(ml_dtypes.bfloat16),
        "pp": pp, "srow": srow, "tio": tio, "kinv": kinv,
    }

    x = inp["input_feature"].astype(np.float32)
    bbox = inp["proposal_bbox"].astype(np.float32)
    cnt = inp["proposal_count"]
    in_maps = []
    for core in range(NCORES):
        xt = np.zeros((V, NCH, 128, TP + 2), ml_dtypes.bfloat16)
        prop = np.zeros((V, 50, 3), np.float32)
        vrow = np.zeros((V, 1, 50), np.float32)
        for j in range(V):
            b = core * V + j
            xT = np.ascontiguousarray(x[b].T).reshape(NCH, 128, T)
            xt[j, :, :, 1:T + 1] = xT.astype(ml_dtypes.bfloat16)
            prop[j, :, 0] = bbox[b, :, 0]
            prop[j, :, 1] = bbox[b, :, 1]
            valid = (np.arange(K) < int(cnt[b])).astype(np.float32)
            prop[j, :, 2] = valid
            vrow[j, 0, :] = valid
        m = dict(shared)
        m.update({"xt": xt, "prop": prop, "vrow": vrow})
        in_maps.append(m)

    global _LAST_IN_MAPS
    _LAST_IN_MAPS = in_maps
    res = run_bass_kernel_spmd(nc, in_maps, core_ids=list(range(NCORES)))

    outs = {k: [] for k in ("o_fg", "o_bg", "o_ta", "o_cas", "o_fgs", "o_bgs", "o_unc")}
    for core in range(NCORES):
        r = res.results[core]
        for j in range(V):
            outs["o_fg"].append(r["o_fg"][j])
            outs["o_bg"].append(r["o_bg"][j])
            outs["o_ta"].append(r["o_ta"][j, :T])
            outs["o_cas"].append(r["o_cas"][j, :T])
            outs["o_fgs"].append(r["o_fgs"][j, :T])
            outs["o_bgs"].append(r["o_bgs"][j, :T])
            outs["o_unc"].append(r["o_unc"][j, :T])
    return tuple(np.stack(outs[k]).astype(np.float32) for k in
                 ("o_fg", "o_bg", "o_ta", "o_cas", "o_fgs", "o_bgs", "o_unc"))
